# revision 12
# baseline (speedup 1.0000x reference)
"""DirConv (bidirectional edge-conditioned GNN conv) on 8 Trainium2 cores.

Strategy (edge-parallel, aggregation-sharded, host-laid-out streams):
  - fwd direction aggregates messages at dst; bwd aggregates at src.
  - Shard each direction's 800K edges across 8 cores by the aggregation
    node's range (12500 nodes per core): every output row is produced by
    exactly one core, no collective needed.
  - Edges are packed into 128-node output windows.  Each core processes
    windows in its own load-sorted order (heaviest first, shared between
    the two directions); the baked per-program-window tile count is the
    max over cores at each rank, which pads much less than aligning
    windows by id.  The host un-permutes the output rows afterward.
  - The host lays out ALL per-edge streams in slot order: edge_attr^T
    (aT, bf16), gathered x rows transposed (xgT, bf16, hid-major), and
    the within-window target row (rel).  The kernel is pure sequential
    streaming - no indirect DMA.
  - Compute chain per 512-slot group (weights fused on host:
    W_em = We2 @ Wm1, bm1c = bm1 + be2 @ Wm1):
      A: h1  = We1^T @ aT               (TensorE, N=512)
         h1r = relu(h1 + be1)           (ScalarE)
         zT  = Wm1^T @ xgT + W_em^T @ h1r   (TensorE)
         rT  = relu(zT + bm1c)          (ScalarE)
      B: m   = rT_j^T @ Wm2s            (TensorE, per tile, msg-major)
         mt  = copy m                   (VectorE)
         S   = one_hot(rel)             (VectorE, batched is_equal)
         out_w += S_j^T @ m_j           (TensorE, per tile)
    B for group g is emitted after A for group g+1 (software pipelining)
    so the in-order TensorE queue never head-of-line blocks on B's
    Vector/Scalar inputs.  Wm2 is pre-scaled by sigmoid(+/-alpha) so the
    direction blend is free; bm2 rides a per-window deg x bm2s rank-1
    matmul.
"""

import numpy as np
import ml_dtypes

import concourse.bass as bass
import concourse.mybir as mybir
import concourse.tile as tile
from concourse.bass_utils import run_bass_kernel_spmd
from concourse.vector_clock import ScopedClock

N_NODES = 100000
N_EDGES = 800000
HID = 128
EDIM = 32
N_CORES = 8
P = 128
NODES_PER_CORE = N_NODES // N_CORES        # 12500
N_WIN = (NODES_PER_CORE + P - 1) // P      # 98
OUT_ROWS = N_WIN * P                       # 12544 (padded)

MM_DT = mybir.dt.bfloat16
MM_NP = ml_dtypes.bfloat16

WCH = 4       # windows per metadata super-chunk
GRP = 4       # tiles per compute macro-group (512-wide ops)


class PatchedTileContext(tile.TileContext):
    """Tail barrier compatible with this container's walrus (one sync-wait
    command per instruction, no eq-mode waits on Drain)."""

    def _drain_and_barrier(self, tick_clock, wait_clock):
        nop = self.nc.sync.nop(nofuse=True)
        wait_clock.add_sem_waits(nop.ins, ScopedClock({None: tick_clock.global_clock}))
        waits = list(nop.ins.sync_info.on_wait) if nop.ins.sync_info else []
        nop.ins.sync_info.on_wait = []
        assert self.sems is not None
        num_to_handle = {h.num: h for h in self.sems.allocated().values()}
        for w in waits:
            h = num_to_handle.get(w.id)
            assert h is not None, f"no handle for sem {w.id} {w.ant_name}"
            self.nc.sync.wait_ge(h, w.wait_value)
        self.nc.sync.drain()
        self.nc._nrt_pseudo_barrier()
        popped = self.nc._tile_sem_poison_stack.pop()
        assert popped is self._sem_poison
        self.nc.clear_and_free_semaphores(list(self.sems.allocated().values()))
        self.nc._nrt_pseudo_barrier()


def _split_multi_waits(nc):
    """Hoist all-but-one sync waits of multi-wait instructions onto dedicated
    single-wait NoOps on the same engine (older walrus allows one wait)."""
    for fn in nc.m.functions:
        for bb in fn.blocks:
            out = []
            dirty = False
            for inst in bb.instructions:
                si = inst.sync_info
                waits = list(si.on_wait) if si is not None else []
                if len(waits) > 1:
                    dirty = True
                    for w in waits[:-1]:
                        out.append(mybir.InstNoOp(
                            name=nc.get_next_instruction_name(),
                            sync_info=mybir.SyncInfo(on_wait=[w], on_update=[]),
                            bass_nofuse=True,
                            engine=inst.engine,
                        ))
                    si.on_wait = [waits[-1]]
                out.append(inst)
            if dirty:
                bb.instructions = out


def _window_orders(counts_f, counts_b):
    """Per-core window processing order: heaviest (f+b) windows first.

    Returns orders [N_CORES, N_WIN] (program slot i -> original window)."""
    total = counts_f + counts_b
    return np.argsort(-total, axis=1, kind="stable")


def _direction_counts(agg):
    agg = np.asarray(agg).astype(np.int64)
    core = agg // NODES_PER_CORE
    local = agg % NODES_PER_CORE
    win = local // P
    counts = np.bincount(core * N_WIN + win, minlength=N_CORES * N_WIN)
    return counts.reshape(N_CORES, N_WIN), core, local


def _prep_direction(core, local, orders, gat, edge_attr, x, counts):
    """Build per-core streams for one direction given the shared window order.

    Returns (k_sched [N_WIN], per-core dict, deg [N_CORES, OUT_ROWS])."""
    # k_sched over program slots: max over cores of that core's i-th window
    ranked = np.take_along_axis(counts, orders, axis=1)   # [C, N_WIN]
    k_sched = np.maximum(1, -(-ranked.max(axis=0) // P))
    win_base_tiles = np.concatenate([[0], np.cumsum(k_sched)[:-1]])
    T = int(k_sched.sum())
    S = T * P

    per_core = []
    deg = np.zeros((N_CORES, OUT_ROWS), dtype=np.float32)
    for c in range(N_CORES):
        pos = np.empty(N_WIN, dtype=np.int64)              # window -> slot
        pos[orders[c]] = np.arange(N_WIN)
        m = np.nonzero(core == c)[0]
        loc = local[m]
        rel = loc % P
        pwin = pos[loc // P]
        order = np.argsort(pwin * P + rel, kind="stable")
        e_sorted = m[order]
        pwin_sorted = pwin[order]
        rel_sorted = rel[order]
        n = len(e_sorted)
        first = np.searchsorted(pwin_sorted, np.arange(N_WIN), side="left")
        rank = np.arange(n) - first[pwin_sorted]
        slots = win_base_tiles[pwin_sorted] * P + rank

        aT = np.zeros((EDIM, S), dtype=MM_NP)
        aT[:, slots] = edge_attr[e_sorted].T.astype(MM_NP)
        xs = np.zeros((S, HID), dtype=np.float32)
        xs[slots] = x[gat[e_sorted]]
        xgT = np.ascontiguousarray(xs.T).astype(MM_NP)      # [HID, S]
        relv = np.full(S, -1.0, dtype=np.float32)
        relv[slots] = rel_sorted.astype(np.float32)
        # deg in program-window order for this core
        dg = np.bincount(loc, minlength=NODES_PER_CORE).astype(np.float32)
        dg = np.concatenate([dg, np.zeros(OUT_ROWS - NODES_PER_CORE, np.float32)])
        deg[c] = dg.reshape(N_WIN, P)[orders[c]].reshape(-1)
        per_core.append({
            "aT": aT,
            "xgT": xgT,
            "rel": relv.reshape(T, P).T.astype(MM_NP).copy(),   # [128, T]
        })
    return k_sched, per_core, deg


def _build_program(k_f, k_b, S_f, S_b):
    nc = bass.Bass("TRN2", target_bir_lowering=False)
    dt = mybir.dt
    f32 = dt.float32

    ins = {}
    for d, S in (("f", S_f), ("b", S_b)):
        T = S // P
        ins[f"aT_{d}"] = nc.dram_tensor(f"aT_{d}", [EDIM, S], MM_DT, kind="ExternalInput")
        ins[f"xgT_{d}"] = nc.dram_tensor(f"xgT_{d}", [HID, S], MM_DT, kind="ExternalInput")
        ins[f"rel_{d}"] = nc.dram_tensor(f"rel_{d}", [P, T], MM_DT, kind="ExternalInput")
        ins[f"We1_{d}"] = nc.dram_tensor(f"We1_{d}", [EDIM, HID], MM_DT, kind="ExternalInput")
        ins[f"Wm1_{d}"] = nc.dram_tensor(f"Wm1_{d}", [HID, HID], MM_DT, kind="ExternalInput")
        ins[f"Wem_{d}"] = nc.dram_tensor(f"Wem_{d}", [HID, HID], MM_DT, kind="ExternalInput")
        ins[f"Wm2_{d}"] = nc.dram_tensor(f"Wm2_{d}", [HID, HID], MM_DT, kind="ExternalInput")
        ins[f"be1_{d}"] = nc.dram_tensor(f"be1_{d}", [HID, 1], f32, kind="ExternalInput")
        ins[f"bm1c_{d}"] = nc.dram_tensor(f"bm1c_{d}", [HID, 1], f32, kind="ExternalInput")
    degterm_d = nc.dram_tensor("degterm", [P, N_WIN * HID], f32, kind="ExternalInput")
    iota_d = nc.dram_tensor("iota", [P, GRP * P], MM_DT, kind="ExternalInput")
    out_d = nc.dram_tensor("out", [OUT_ROWS, HID], f32, kind="ExternalOutput")

    ks = {"f": k_f, "b": k_b}
    tile_base = {"f": np.concatenate([[0], np.cumsum(k_f)[:-1]]),
                 "b": np.concatenate([[0], np.cumsum(k_b)[:-1]])}
    km = int(max(k_f.max(), k_b.max()))
    relu = mybir.ActivationFunctionType.Relu

    # flat group list: (w, d, g0, g, first_of_wd, last_of_wd)
    groups = []
    for w in range(N_WIN):
        for d in ("f", "b"):
            kw = int(ks[d][w])
            for g0 in range(0, kw, GRP):
                g = min(GRP, kw - g0)
                groups.append((w, d, g0, g, g0 == 0, g0 + g >= kw))

    with PatchedTileContext(nc) as tc:
        with (
            tc.tile_pool(name="const", bufs=1) as cpool,
            tc.tile_pool(name="meta", bufs=2) as mpool,
            tc.tile_pool(name="work", bufs=6) as wpool,
            tc.tile_pool(name="ps_h1", bufs=2, space="PSUM") as ph1,
            tc.tile_pool(name="ps_z", bufs=2, space="PSUM") as pz,
            tc.tile_pool(name="ps_m", bufs=2, space="PSUM") as pm,
            tc.tile_pool(name="ps_out", bufs=2, space="PSUM") as pout,
        ):
            # ---- constants / weights ----
            iota_t = cpool.tile([P, GRP * P], MM_DT)
            nc.sync.dma_start(out=iota_t[:], in_=iota_d[:])

            W = {}
            bias = {}
            for i, d in enumerate(("f", "b")):
                for wn, pdim in (("We1", EDIM), ("Wm1", HID), ("Wem", HID), ("Wm2", HID)):
                    t = cpool.tile([pdim, HID], MM_DT, tag=f"{wn}_{d}")
                    nc.sync.dma_start(out=t[:], in_=ins[f"{wn}_{d}"][:])
                    W[f"{wn}_{d}"] = t
                for bn in ("be1", "bm1c"):
                    t = cpool.tile([HID, 1], f32, tag=f"{bn}_{d}")
                    nc.sync.dma_start(out=t[:], in_=ins[f"{bn}_{d}"][:])
                    bias[f"{bn}_{d}"] = t

            chunk_tiles = {}
            dt_tiles = {}        # chunk idx -> degterm tile
            state = {}           # per live group: tiles for the B stage
            win_state = {}       # w -> [ps_out tile, agg matmul count]

            def stage_a1(gi):
                w, d, g0, g, first_wd, last_wd = groups[gi]
                if w % WCH == 0 and first_wd:
                    we = min(w + WCH, N_WIN)
                    c0 = int(tile_base[d][w])
                    c1 = int(tile_base[d][we - 1] + ks[d][we - 1])
                    ck = c1 - c0
                    aT_c = mpool.tile([EDIM, km * WCH * P], MM_DT, tag="aT_c")
                    nc.sync.dma_start(out=aT_c[:, :ck * P],
                                      in_=ins[f"aT_{d}"][:, c0 * P:c1 * P])
                    xgT_c = mpool.tile([HID, km * WCH * P], MM_DT, tag="xgT_c")
                    nc.sync.dma_start(out=xgT_c[:, :ck * P],
                                      in_=ins[f"xgT_{d}"][:, c0 * P:c1 * P])
                    rel_c = mpool.tile([P, km * WCH], MM_DT, tag="rel_c")
                    nc.sync.dma_start(out=rel_c[:, :ck], in_=ins[f"rel_{d}"][:, c0:c1])
                    chunk_tiles[d] = (aT_c, xgT_c, rel_c, c0)
                    if d == "f":
                        dt_c = mpool.tile([P, WCH * HID], f32, tag="dt_c")
                        nc.sync.dma_start(out=dt_c[:, :(we - w) * HID],
                                          in_=degterm_d[:, w * HID:we * HID])
                        dt_tiles[w // WCH] = dt_c
                aT_full, xgT_full, rel_full, c0 = chunk_tiles[d]
                lt = int(tile_base[d][w]) - c0
                gw = g * P
                csl = slice((lt + g0) * P, (lt + g0 + g) * P)
                ps_h1 = ph1.tile([HID, GRP * P], f32, tag="ps_h1")
                nc.tensor.matmul(out=ps_h1[:, :gw], lhsT=W[f"We1_{d}"][:],
                                 rhs=aT_full[:, csl], start=True, stop=True)
                h1r = wpool.tile([HID, GRP * P], MM_DT, tag="h1r")
                nc.scalar.activation(h1r[:, :gw], ps_h1[:, :gw], func=relu,
                                     bias=bias[f"be1_{d}"][:])
                ps_z = pz.tile([HID, GRP * P], f32, tag="ps_z")
                nc.tensor.matmul(out=ps_z[:, :gw], lhsT=W[f"Wm1_{d}"][:],
                                 rhs=xgT_full[:, csl], start=True, stop=False)
                state[gi] = [None, rel_full, lt, h1r, ps_z]

            def stage_a2(gi):
                w, d, g0, g, first_wd, last_wd = groups[gi]
                _, rel_full, lt, h1r, ps_z = state[gi]
                gw = g * P
                nc.tensor.matmul(out=ps_z[:, :gw], lhsT=W[f"Wem_{d}"][:],
                                 rhs=h1r[:, :gw], start=False, stop=True)
                rT = wpool.tile([HID, GRP * P], MM_DT, tag="rT")
                nc.scalar.activation(rT[:, :gw], ps_z[:, :gw], func=relu,
                                     bias=bias[f"bm1c_{d}"][:])
                state[gi] = [rT, rel_full, lt, None, None]

            def stage_b(gi):
                w, d, g0, g, first_wd, last_wd = groups[gi]
                rT, rel_full, lt, _, _ = state.pop(gi)
                gw = g * P
                if w not in win_state:
                    ps_out_t = pout.tile([P, HID], f32, tag="ps_out")
                    win_state[w] = [ps_out_t, 0]
                ps_out, mm_i = win_state[w]
                S_t = wpool.tile([P, GRP * P], MM_DT, tag="S")
                nc.vector.tensor_tensor(
                    out=S_t[:, :gw],
                    in0=rel_full[:, lt + g0:lt + g0 + g].to_broadcast([P, g, P]),
                    in1=iota_t[:, :gw], op=mybir.AluOpType.is_equal)
                ps_m = pm.tile([P, GRP * HID], f32, tag="ps_m")
                for j in range(g):
                    nc.tensor.matmul(out=ps_m[:, j * HID:(j + 1) * HID],
                                     lhsT=rT[:, j * P:(j + 1) * P],
                                     rhs=W[f"Wm2_{d}"][:], start=True, stop=True)
                mt = wpool.tile([P, GRP * HID], MM_DT, tag="mt")
                nc.vector.tensor_copy(out=mt[:, :gw], in_=ps_m[:, :gw])
                last_of_win = last_wd and d == "b"
                for j in range(g):
                    nc.tensor.matmul(out=ps_out[:],
                                     lhsT=S_t[:, j * P:(j + 1) * P],
                                     rhs=mt[:, j * HID:(j + 1) * HID],
                                     start=(mm_i == 0),
                                     stop=(last_of_win and j == g - 1))
                    mm_i += 1
                win_state[w][1] = mm_i
                if last_of_win:
                    dt_c = dt_tiles[w // WCH]
                    stage = wpool.tile([P, HID], f32, tag="stage")
                    nc.vector.tensor_tensor(
                        out=stage[:], in0=ps_out[:],
                        in1=dt_c[:, (w % WCH) * HID:(w % WCH + 1) * HID],
                        op=mybir.AluOpType.add)
                    nc.scalar.dma_start(out=out_d[w * P:(w + 1) * P, :], in_=stage[:])
                    del win_state[w]

            # software pipeline: emit A1(g), B(g-2), A2(g)
            n = len(groups)
            for gi in range(n):
                stage_a1(gi)
                if gi >= 2:
                    stage_b(gi - 2)
                stage_a2(gi)
            stage_b(n - 2)
            stage_b(n - 1)

    _split_multi_waits(nc)
    from concourse.library_overlay import lower_extended_insts
    lower_extended_insts(nc)
    return nc


def kernel(x, edge_index, edge_attr,
           f_We1, f_be1, f_We2, f_be2, f_Wm1, f_bm1, f_Wm2, f_bm2,
           b_We1, b_be1, b_We2, b_be2, b_Wm1, b_bm1, b_Wm2, b_bm2,
           alpha):
    x = np.asarray(x, dtype=np.float32)
    edge_index = np.asarray(edge_index)
    edge_attr = np.asarray(edge_attr, dtype=np.float32)
    src, dst = edge_index[0], edge_index[1]

    counts_f, core_f, local_f = _direction_counts(dst)   # fwd: agg at dst
    counts_b, core_b, local_b = _direction_counts(src)   # bwd: agg at src
    orders = _window_orders(counts_f, counts_b)

    gat_f = np.asarray(src).astype(np.int64)
    gat_b = np.asarray(dst).astype(np.int64)
    k_f, pc_f, deg_f = _prep_direction(core_f, local_f, orders, gat_f,
                                       edge_attr, x, counts_f)
    k_b, pc_b, deg_b = _prep_direction(core_b, local_b, orders, gat_b,
                                       edge_attr, x, counts_b)
    S_f = int(k_f.sum()) * P
    S_b = int(k_b.sum()) * P

    nc = _build_program(k_f, k_b, S_f, S_b)

    weights = {
        "f": (f_We1, f_be1, f_We2, f_be2, f_Wm1, f_bm1, f_Wm2, f_bm2),
        "b": (b_We1, b_be1, b_We2, b_be2, b_Wm1, b_bm1, b_Wm2, b_bm2),
    }
    alpha_f = float(np.asarray(alpha))
    sig = {"f": 1.0 / (1.0 + np.exp(-alpha_f)),
           "b": 1.0 / (1.0 + np.exp(alpha_f))}
    iota = np.broadcast_to(np.arange(P, dtype=np.float32), (P, P))
    iota = np.tile(iota, (1, GRP)).astype(MM_NP)

    fused = {}
    bm2s = {}
    for d in ("f", "b"):
        We1, be1, We2, be2, Wm1, bm1, Wm2, bm2 = [
            np.asarray(a, dtype=np.float32) for a in weights[d]]
        bm2s[d] = bm2.reshape(HID) * sig[d]
        fused[d] = {
            f"We1_{d}": We1.astype(MM_NP),
            f"Wm1_{d}": Wm1.astype(MM_NP),
            f"Wem_{d}": (We2 @ Wm1).astype(MM_NP),
            f"Wm2_{d}": (Wm2 * sig[d]).astype(MM_NP),
            f"be1_{d}": be1.reshape(HID, 1),
            f"bm1c_{d}": (bm1 + be2 @ Wm1).reshape(HID, 1),
        }

    in_maps = []
    for c in range(N_CORES):
        # degterm[r, w*HID:(w+1)*HID] = deg_f[w,r]*bm2s_f + deg_b[w,r]*bm2s_b
        dterm = (deg_f[c].reshape(N_WIN, P)[:, :, None] * bm2s["f"][None, None, :]
                 + deg_b[c].reshape(N_WIN, P)[:, :, None] * bm2s["b"][None, None, :])
        degterm = np.ascontiguousarray(
            dterm.transpose(1, 0, 2).reshape(P, N_WIN * HID)).astype(np.float32)
        m = {"iota": iota, "degterm": degterm}
        for d, pc in (("f", pc_f), ("b", pc_b)):
            m[f"aT_{d}"] = pc[c]["aT"]
            m[f"xgT_{d}"] = pc[c]["xgT"]
            m[f"rel_{d}"] = pc[c]["rel"]
            m.update(fused[d])
        in_maps.append(m)

    import time as _time
    _t0 = _time.time()
    res = run_bass_kernel_spmd(nc, in_maps, core_ids=list(range(N_CORES)))
    globals()["LAST_EXEC_WALL_NS"] = int((_time.time() - _t0) * 1e9)

    out = np.empty((N_NODES, HID), dtype=np.float32)
    for c in range(N_CORES):
        rows = res.results[c]["out"].reshape(N_WIN, P, HID)
        unperm = np.empty_like(rows)
        unperm[orders[c]] = rows
        out[c * NODES_PER_CORE:(c + 1) * NODES_PER_CORE] = \
            unperm.reshape(OUT_ROWS, HID)[:NODES_PER_CORE]
    return out.astype(np.float32)


# revision 15
# speedup vs baseline: 1.2180x; 1.2180x over previous
"""DirConv (bidirectional edge-conditioned GNN conv) on 8 Trainium2 cores.

Strategy (edge-parallel, aggregation-sharded, host-laid-out streams):
  - fwd direction aggregates messages at dst; bwd aggregates at src.
  - Shard each direction's 800K edges across 8 cores by the aggregation
    node's range (12500 nodes per core): every output row is produced by
    exactly one core, no collective needed.
  - Edges are packed into 128-node output windows.  Each core processes
    windows in its own load-sorted order (heaviest first, shared between
    the two directions); the baked per-program-window tile count is the
    max over cores at each rank, which pads much less than aligning
    windows by id.  The host un-permutes the output rows afterward.
  - The host lays out ALL per-edge streams in slot order: edge_attr^T
    (aT, bf16), gathered x rows transposed (xgT, bf16, hid-major), and
    the within-window target row (rel).  The kernel is pure sequential
    streaming - no indirect DMA.
  - Compute chain per 512-slot group (weights fused on host:
    W_em = We2 @ Wm1, bm1c = bm1 + be2 @ Wm1):
      A: h1  = We1^T @ aT               (TensorE, N=512)
         h1r = relu(h1 + be1)           (ScalarE)
         zT  = Wm1^T @ xgT + W_em^T @ h1r   (TensorE)
         rT  = relu(zT + bm1c)          (ScalarE)
      B: m   = rT_j^T @ Wm2s            (TensorE, per tile, msg-major)
         mt  = copy m                   (VectorE)
         S   = one_hot(rel)             (VectorE, batched is_equal)
         out_w += S_j^T @ m_j           (TensorE, per tile)
    B for group g is emitted after A for group g+1 (software pipelining)
    so the in-order TensorE queue never head-of-line blocks on B's
    Vector/Scalar inputs.  Wm2 is pre-scaled by sigmoid(+/-alpha) so the
    direction blend is free; bm2 rides a per-window deg x bm2s rank-1
    matmul.
"""

import numpy as np
import ml_dtypes

import concourse.bass as bass
import concourse.mybir as mybir
import concourse.tile as tile
from concourse.bass_utils import run_bass_kernel_spmd
from concourse.vector_clock import ScopedClock

N_NODES = 100000
N_EDGES = 800000
HID = 128
EDIM = 32
N_CORES = 8
P = 128
NODES_PER_CORE = N_NODES // N_CORES        # 12500
N_WIN = (NODES_PER_CORE + P - 1) // P      # 98
OUT_ROWS = N_WIN * P                       # 12544 (padded)

MM_DT = mybir.dt.bfloat16
MM_NP = ml_dtypes.bfloat16

WCH = 4       # windows per metadata super-chunk
GRP = 4       # tiles per compute macro-group (512-wide ops)


class PatchedTileContext(tile.TileContext):
    """Tail barrier compatible with this container's walrus (one sync-wait
    command per instruction, no eq-mode waits on Drain)."""

    def _drain_and_barrier(self, tick_clock, wait_clock):
        nop = self.nc.sync.nop(nofuse=True)
        wait_clock.add_sem_waits(nop.ins, ScopedClock({None: tick_clock.global_clock}))
        waits = list(nop.ins.sync_info.on_wait) if nop.ins.sync_info else []
        nop.ins.sync_info.on_wait = []
        assert self.sems is not None
        num_to_handle = {h.num: h for h in self.sems.allocated().values()}
        for w in waits:
            h = num_to_handle.get(w.id)
            assert h is not None, f"no handle for sem {w.id} {w.ant_name}"
            self.nc.sync.wait_ge(h, w.wait_value)
        self.nc.sync.drain()
        self.nc._nrt_pseudo_barrier()
        popped = self.nc._tile_sem_poison_stack.pop()
        assert popped is self._sem_poison
        self.nc.clear_and_free_semaphores(list(self.sems.allocated().values()))
        self.nc._nrt_pseudo_barrier()


def _split_multi_waits(nc):
    """Hoist all-but-one sync waits of multi-wait instructions onto dedicated
    single-wait NoOps on the same engine (older walrus allows one wait)."""
    for fn in nc.m.functions:
        for bb in fn.blocks:
            out = []
            dirty = False
            for inst in bb.instructions:
                si = inst.sync_info
                waits = list(si.on_wait) if si is not None else []
                if len(waits) > 1:
                    dirty = True
                    for w in waits[:-1]:
                        out.append(mybir.InstNoOp(
                            name=nc.get_next_instruction_name(),
                            sync_info=mybir.SyncInfo(on_wait=[w], on_update=[]),
                            bass_nofuse=True,
                            engine=inst.engine,
                        ))
                    si.on_wait = [waits[-1]]
                out.append(inst)
            if dirty:
                bb.instructions = out


def _window_orders(counts_f, counts_b):
    """Per-core window processing order: heaviest (f+b) windows first.

    Returns orders [N_CORES, N_WIN] (program slot i -> original window)."""
    total = counts_f + counts_b
    return np.argsort(-total, axis=1, kind="stable")


def _direction_counts(agg):
    agg = np.asarray(agg).astype(np.int64)
    core = agg // NODES_PER_CORE
    local = agg % NODES_PER_CORE
    win = local // P
    counts = np.bincount(core * N_WIN + win, minlength=N_CORES * N_WIN)
    return counts.reshape(N_CORES, N_WIN), core, local


def _prep_direction(core, local, orders, gat, edge_attr, x, counts):
    """Build per-core streams for one direction given the shared window order.

    Returns (k_sched [N_WIN], per-core dict, deg [N_CORES, OUT_ROWS])."""
    # k_sched over program slots: max over cores of that core's i-th window
    ranked = np.take_along_axis(counts, orders, axis=1)   # [C, N_WIN]
    k_sched = np.maximum(1, -(-ranked.max(axis=0) // P))
    win_base_tiles = np.concatenate([[0], np.cumsum(k_sched)[:-1]])
    T = int(k_sched.sum())
    S = T * P

    per_core = []
    deg = np.zeros((N_CORES, OUT_ROWS), dtype=np.float32)
    for c in range(N_CORES):
        pos = np.empty(N_WIN, dtype=np.int64)              # window -> slot
        pos[orders[c]] = np.arange(N_WIN)
        m = np.nonzero(core == c)[0]
        loc = local[m]
        rel = loc % P
        pwin = pos[loc // P]
        order = np.argsort(pwin * P + rel, kind="stable")
        e_sorted = m[order]
        pwin_sorted = pwin[order]
        rel_sorted = rel[order]
        n = len(e_sorted)
        first = np.searchsorted(pwin_sorted, np.arange(N_WIN), side="left")
        rank = np.arange(n) - first[pwin_sorted]
        slots = win_base_tiles[pwin_sorted] * P + rank

        aT = np.zeros((EDIM, S), dtype=MM_NP)
        aT[:, slots] = edge_attr[e_sorted].T.astype(MM_NP)
        xs = np.zeros((S, HID), dtype=np.float32)
        xs[slots] = x[gat[e_sorted]]
        xgT = np.ascontiguousarray(xs.T).astype(MM_NP)      # [HID, S]
        relv = np.full(S, -1.0, dtype=np.float32)
        relv[slots] = rel_sorted.astype(np.float32)
        # deg in program-window order for this core
        dg = np.bincount(loc, minlength=NODES_PER_CORE).astype(np.float32)
        dg = np.concatenate([dg, np.zeros(OUT_ROWS - NODES_PER_CORE, np.float32)])
        deg[c] = dg.reshape(N_WIN, P)[orders[c]].reshape(-1)
        per_core.append({
            "aT": aT,
            "xgT": xgT,
            "rel": relv.reshape(T, P).T.astype(MM_NP).copy(),   # [128, T]
        })
    return k_sched, per_core, deg


def _build_program(k_f, k_b, S_f, S_b):
    nc = bass.Bass("TRN2", target_bir_lowering=False)
    dt = mybir.dt
    f32 = dt.float32

    ins = {}
    for d, S in (("f", S_f), ("b", S_b)):
        T = S // P
        ins[f"aT_{d}"] = nc.dram_tensor(f"aT_{d}", [EDIM, S], MM_DT, kind="ExternalInput")
        ins[f"xgT_{d}"] = nc.dram_tensor(f"xgT_{d}", [HID, S], MM_DT, kind="ExternalInput")
        ins[f"rel_{d}"] = nc.dram_tensor(f"rel_{d}", [P, T], MM_DT, kind="ExternalInput")
        ins[f"We1_{d}"] = nc.dram_tensor(f"We1_{d}", [EDIM, HID], MM_DT, kind="ExternalInput")
        ins[f"Wm1_{d}"] = nc.dram_tensor(f"Wm1_{d}", [HID, HID], MM_DT, kind="ExternalInput")
        ins[f"Wem_{d}"] = nc.dram_tensor(f"Wem_{d}", [HID, HID], MM_DT, kind="ExternalInput")
        ins[f"Wm2_{d}"] = nc.dram_tensor(f"Wm2_{d}", [HID, HID], MM_DT, kind="ExternalInput")
        ins[f"be1_{d}"] = nc.dram_tensor(f"be1_{d}", [HID, 1], f32, kind="ExternalInput")
        ins[f"bm1c_{d}"] = nc.dram_tensor(f"bm1c_{d}", [HID, 1], f32, kind="ExternalInput")
    degterm_d = nc.dram_tensor("degterm", [P, N_WIN * HID], f32, kind="ExternalInput")
    iota_d = nc.dram_tensor("iota", [P, GRP * P], MM_DT, kind="ExternalInput")
    out_d = nc.dram_tensor("out", [OUT_ROWS, HID], f32, kind="ExternalOutput")

    ks = {"f": k_f, "b": k_b}
    tile_base = {"f": np.concatenate([[0], np.cumsum(k_f)[:-1]]),
                 "b": np.concatenate([[0], np.cumsum(k_b)[:-1]])}
    km = int(max(k_f.max(), k_b.max()))
    relu = mybir.ActivationFunctionType.Relu

    # flat group list: (w, d, g0, g, first_of_wd, last_of_wd)
    groups = []
    for w in range(N_WIN):
        for d in ("f", "b"):
            kw = int(ks[d][w])
            for g0 in range(0, kw, GRP):
                g = min(GRP, kw - g0)
                groups.append((w, d, g0, g, g0 == 0, g0 + g >= kw))

    with PatchedTileContext(nc) as tc:
        with (
            tc.tile_pool(name="const", bufs=1) as cpool,
            tc.tile_pool(name="meta", bufs=3) as mpool,
            tc.tile_pool(name="work", bufs=6) as wpool,
            tc.tile_pool(name="ps_h1", bufs=2, space="PSUM") as ph1,
            tc.tile_pool(name="ps_z", bufs=2, space="PSUM") as pz,
            tc.tile_pool(name="ps_m", bufs=2, space="PSUM") as pm,
            tc.tile_pool(name="ps_out", bufs=2, space="PSUM") as pout,
        ):
            # ---- constants / weights ----
            iota_t = cpool.tile([P, GRP * P], MM_DT)
            nc.sync.dma_start(out=iota_t[:], in_=iota_d[:])

            W = {}
            bias = {}
            for i, d in enumerate(("f", "b")):
                for wn, pdim in (("We1", EDIM), ("Wm1", HID), ("Wem", HID), ("Wm2", HID)):
                    t = cpool.tile([pdim, HID], MM_DT, tag=f"{wn}_{d}")
                    nc.sync.dma_start(out=t[:], in_=ins[f"{wn}_{d}"][:])
                    W[f"{wn}_{d}"] = t
                for bn in ("be1", "bm1c"):
                    t = cpool.tile([HID, 1], f32, tag=f"{bn}_{d}")
                    nc.sync.dma_start(out=t[:], in_=ins[f"{bn}_{d}"][:])
                    bias[f"{bn}_{d}"] = t

            chunk_tiles = {}
            dt_tiles = {}        # chunk idx -> degterm tile
            state = {}           # per live group: tiles for the B stage
            win_state = {}       # w -> [ps_out tile, agg matmul count]

            def stage_a1(gi):
                w, d, g0, g, first_wd, last_wd = groups[gi]
                if w % WCH == 0 and first_wd:
                    we = min(w + WCH, N_WIN)
                    c0 = int(tile_base[d][w])
                    c1 = int(tile_base[d][we - 1] + ks[d][we - 1])
                    ck = c1 - c0
                    aT_c = mpool.tile([EDIM, km * WCH * P], MM_DT, tag="aT_c")
                    nc.sync.dma_start(out=aT_c[:, :ck * P],
                                      in_=ins[f"aT_{d}"][:, c0 * P:c1 * P])
                    xgT_c = mpool.tile([HID, km * WCH * P], MM_DT, tag="xgT_c")
                    nc.sync.dma_start(out=xgT_c[:, :ck * P],
                                      in_=ins[f"xgT_{d}"][:, c0 * P:c1 * P])
                    rel_c = mpool.tile([P, km * WCH], MM_DT, tag="rel_c")
                    nc.sync.dma_start(out=rel_c[:, :ck], in_=ins[f"rel_{d}"][:, c0:c1])
                    chunk_tiles[d] = (aT_c, xgT_c, rel_c, c0)
                    if d == "f":
                        dt_c = mpool.tile([P, WCH * HID], f32, tag="dt_c")
                        nc.sync.dma_start(out=dt_c[:, :(we - w) * HID],
                                          in_=degterm_d[:, w * HID:we * HID])
                        dt_tiles[w // WCH] = dt_c
                aT_full, xgT_full, rel_full, c0 = chunk_tiles[d]
                lt = int(tile_base[d][w]) - c0
                gw = g * P
                csl = slice((lt + g0) * P, (lt + g0 + g) * P)
                ps_h1 = ph1.tile([HID, GRP * P], f32, tag="ps_h1")
                nc.tensor.matmul(out=ps_h1[:, :gw], lhsT=W[f"We1_{d}"][:],
                                 rhs=aT_full[:, csl], start=True, stop=True)
                h1r = wpool.tile([HID, GRP * P], MM_DT, tag="h1r")
                nc.scalar.activation(h1r[:, :gw], ps_h1[:, :gw], func=relu,
                                     bias=bias[f"be1_{d}"][:])
                ps_z = pz.tile([HID, GRP * P], f32, tag="ps_z")
                nc.tensor.matmul(out=ps_z[:, :gw], lhsT=W[f"Wm1_{d}"][:],
                                 rhs=xgT_full[:, csl], start=True, stop=False)
                state[gi] = [None, rel_full, lt, h1r, ps_z]

            def stage_a2(gi):
                w, d, g0, g, first_wd, last_wd = groups[gi]
                _, rel_full, lt, h1r, ps_z = state[gi]
                gw = g * P
                nc.tensor.matmul(out=ps_z[:, :gw], lhsT=W[f"Wem_{d}"][:],
                                 rhs=h1r[:, :gw], start=False, stop=True)
                rT = wpool.tile([HID, GRP * P], MM_DT, tag="rT")
                nc.scalar.activation(rT[:, :gw], ps_z[:, :gw], func=relu,
                                     bias=bias[f"bm1c_{d}"][:])
                state[gi] = [rT, rel_full, lt, None, None]

            def stage_b1(gi):
                w, d, g0, g, first_wd, last_wd = groups[gi]
                st = state[gi]
                rT, rel_full, lt = st[0], st[1], st[2]
                gw = g * P
                S_t = wpool.tile([P, GRP * P], MM_DT, tag="S")
                nc.vector.tensor_tensor(
                    out=S_t[:, :gw],
                    in0=rel_full[:, lt + g0:lt + g0 + g].to_broadcast([P, g, P]),
                    in1=iota_t[:, :gw], op=mybir.AluOpType.is_equal)
                ps_m = pm.tile([P, GRP * HID], f32, tag="ps_m")
                for j in range(g):
                    nc.tensor.matmul(out=ps_m[:, j * HID:(j + 1) * HID],
                                     lhsT=rT[:, j * P:(j + 1) * P],
                                     rhs=W[f"Wm2_{d}"][:], start=True, stop=True)
                mt = wpool.tile([P, GRP * HID], MM_DT, tag="mt")
                nc.vector.tensor_copy(out=mt[:, :gw], in_=ps_m[:, :gw])
                state[gi] = [S_t, mt, None]

            def stage_b2(gi):
                w, d, g0, g, first_wd, last_wd = groups[gi]
                S_t, mt, _ = state.pop(gi)
                gw = g * P
                if w not in win_state:
                    ps_out_t = pout.tile([P, HID], f32, tag="ps_out")
                    win_state[w] = [ps_out_t, 0]
                ps_out, mm_i = win_state[w]
                last_of_win = last_wd and d == "b"
                for j in range(g):
                    nc.tensor.matmul(out=ps_out[:],
                                     lhsT=S_t[:, j * P:(j + 1) * P],
                                     rhs=mt[:, j * HID:(j + 1) * HID],
                                     start=(mm_i == 0),
                                     stop=(last_of_win and j == g - 1))
                    mm_i += 1
                win_state[w][1] = mm_i
                if last_of_win:
                    dt_c = dt_tiles[w // WCH]
                    stage = wpool.tile([P, HID], f32, tag="stage")
                    nc.vector.tensor_tensor(
                        out=stage[:], in0=ps_out[:],
                        in1=dt_c[:, (w % WCH) * HID:(w % WCH + 1) * HID],
                        op=mybir.AluOpType.add)
                    nc.scalar.dma_start(out=out_d[w * P:(w + 1) * P, :], in_=stage[:])
                    del win_state[w]

            # software pipeline: A1(g), B1(g-2), A2(g), B2(g-3)
            n = len(groups)
            for gi in range(n):
                stage_a1(gi)
                if gi >= 2:
                    stage_b1(gi - 2)
                stage_a2(gi)
                if gi >= 3:
                    stage_b2(gi - 3)
            stage_b1(n - 2)
            stage_b2(n - 3)
            stage_b1(n - 1)
            stage_b2(n - 2)
            stage_b2(n - 1)

    _split_multi_waits(nc)
    from concourse.library_overlay import lower_extended_insts
    lower_extended_insts(nc)
    return nc


def kernel(x, edge_index, edge_attr,
           f_We1, f_be1, f_We2, f_be2, f_Wm1, f_bm1, f_Wm2, f_bm2,
           b_We1, b_be1, b_We2, b_be2, b_Wm1, b_bm1, b_Wm2, b_bm2,
           alpha):
    x = np.asarray(x, dtype=np.float32)
    edge_index = np.asarray(edge_index)
    edge_attr = np.asarray(edge_attr, dtype=np.float32)
    src, dst = edge_index[0], edge_index[1]

    counts_f, core_f, local_f = _direction_counts(dst)   # fwd: agg at dst
    counts_b, core_b, local_b = _direction_counts(src)   # bwd: agg at src
    orders = _window_orders(counts_f, counts_b)

    gat_f = np.asarray(src).astype(np.int64)
    gat_b = np.asarray(dst).astype(np.int64)
    k_f, pc_f, deg_f = _prep_direction(core_f, local_f, orders, gat_f,
                                       edge_attr, x, counts_f)
    k_b, pc_b, deg_b = _prep_direction(core_b, local_b, orders, gat_b,
                                       edge_attr, x, counts_b)
    S_f = int(k_f.sum()) * P
    S_b = int(k_b.sum()) * P

    nc = _build_program(k_f, k_b, S_f, S_b)

    weights = {
        "f": (f_We1, f_be1, f_We2, f_be2, f_Wm1, f_bm1, f_Wm2, f_bm2),
        "b": (b_We1, b_be1, b_We2, b_be2, b_Wm1, b_bm1, b_Wm2, b_bm2),
    }
    alpha_f = float(np.asarray(alpha))
    sig = {"f": 1.0 / (1.0 + np.exp(-alpha_f)),
           "b": 1.0 / (1.0 + np.exp(alpha_f))}
    iota = np.broadcast_to(np.arange(P, dtype=np.float32), (P, P))
    iota = np.tile(iota, (1, GRP)).astype(MM_NP)

    fused = {}
    bm2s = {}
    for d in ("f", "b"):
        We1, be1, We2, be2, Wm1, bm1, Wm2, bm2 = [
            np.asarray(a, dtype=np.float32) for a in weights[d]]
        bm2s[d] = bm2.reshape(HID) * sig[d]
        fused[d] = {
            f"We1_{d}": We1.astype(MM_NP),
            f"Wm1_{d}": Wm1.astype(MM_NP),
            f"Wem_{d}": (We2 @ Wm1).astype(MM_NP),
            f"Wm2_{d}": (Wm2 * sig[d]).astype(MM_NP),
            f"be1_{d}": be1.reshape(HID, 1),
            f"bm1c_{d}": (bm1 + be2 @ Wm1).reshape(HID, 1),
        }

    in_maps = []
    for c in range(N_CORES):
        # degterm[r, w*HID:(w+1)*HID] = deg_f[w,r]*bm2s_f + deg_b[w,r]*bm2s_b
        dterm = (deg_f[c].reshape(N_WIN, P)[:, :, None] * bm2s["f"][None, None, :]
                 + deg_b[c].reshape(N_WIN, P)[:, :, None] * bm2s["b"][None, None, :])
        degterm = np.ascontiguousarray(
            dterm.transpose(1, 0, 2).reshape(P, N_WIN * HID)).astype(np.float32)
        m = {"iota": iota, "degterm": degterm}
        for d, pc in (("f", pc_f), ("b", pc_b)):
            m[f"aT_{d}"] = pc[c]["aT"]
            m[f"xgT_{d}"] = pc[c]["xgT"]
            m[f"rel_{d}"] = pc[c]["rel"]
            m.update(fused[d])
        in_maps.append(m)

    import time as _time
    _t0 = _time.time()
    res = run_bass_kernel_spmd(nc, in_maps, core_ids=list(range(N_CORES)))
    globals()["LAST_EXEC_WALL_NS"] = int((_time.time() - _t0) * 1e9)

    out = np.empty((N_NODES, HID), dtype=np.float32)
    for c in range(N_CORES):
        rows = res.results[c]["out"].reshape(N_WIN, P, HID)
        unperm = np.empty_like(rows)
        unperm[orders[c]] = rows
        out[c * NODES_PER_CORE:(c + 1) * NODES_PER_CORE] = \
            unperm.reshape(OUT_ROWS, HID)[:NODES_PER_CORE]
    return out.astype(np.float32)
